# revision 27
# baseline (speedup 1.0000x reference)
"""MiniMax-M2 decoder layer (attention + sigmoid-router top-2 MoE) on 8 TRN2 NeuronCores.

Sharding: head-parallel attention (2 q-heads + 1 kv-head per core), token-parallel
everything else (256 tokens per core).  Collectives: AllGather of x^T (+ln1 rms row),
AllReduce of qk-norm sumsq partials, AllToAll to reshard o^T from head- to
token-sharded, then expert-parallel MoE: 2x half-capacity AllToAll dispatch of
top-2-routed tokens (capacity 128 slots per (src core, expert)), one expert's GLU
per core in bf16, 2x half-D AllToAll combine, and affinity-weighted scatter-add
back into token-major output.  Router in hi/lo split fp32r (~1e-7) to keep top-2
selection faithful.  Partition broadcasts are done as rank-1 matmuls; the MoE
weights prefetch early on the scalar HWDGE queue.

kernel(**inputs) takes full unsharded inputs, returns the full [1, S, D] output.
"""

import contextlib

import numpy as np
import ml_dtypes

import concourse.bass as bass
import concourse.mybir as mybir
import concourse.tile as tile
from concourse import bacc, bass_isa, bass_utils

F32 = mybir.dt.float32
F32R = mybir.dt.float32r
BF16 = mybir.dt.bfloat16
AF = mybir.ActivationFunctionType
OP = mybir.AluOpType
AX = mybir.AxisListType
RG8 = [list(range(8))]

P = 128
D = 2048
H = 16
KVH = 4
DH = 128
RD = 64
E = 8
I = 1024
S = 2048
NCORE = 8
TPC = S // NCORE          # 256 tokens per core
HPC = H // NCORE          # 2 q-heads per core
DKT = D // P              # 16
IKT = I // P              # 8
IMT = I // P              # 8
DMT = D // P              # 16
NCH = S // 512            # 4 q-chunks
CAP = 128                 # MoE capacity per (src core, expert)
HCAP = CAP // 2           # dispatched in two half-capacity AllToAlls
SLOTS = NCORE * CAP       # 1024 dispatch slots per expert core
EPS = 1e-6
ISQ_DH = float(1.0 / np.sqrt(DH))


# ======================================================================
# device program
# ======================================================================

def build_module(dbg=False):
    nc = bacc.Bacc("TRN2", target_bir_lowering=False, debug=False, num_devices=NCORE)

    def inp(name, shape, dt):
        return nc.dram_tensor(name, list(shape), dt, kind="ExternalInput")

    x_sl = inp("x_sl", [TPC, D], F32R)
    wqkv = inp("wqkv", [P, 24, DKT, P], BF16)        # full q|k|v lhsT tiles [p, m, kd, col]
    wo_p = inp("wo_p", [P, DMT, DKT, P], BF16)       # packed lhsT tiles
    rwh_in = inp("rwh_in", [P, DKT, E], F32R)
    rwl_in = inp("rwl_in", [P, DKT, E], F32R)
    cos_in = inp("cos_in", [RD, S], BF16)
    sin_in = inp("sin_in", [RD, S], BF16)
    id_r = inp("id_r", [P, P], F32R)
    id_f = inp("id_f", [P, P], F32)
    id_b = inp("id_b", [P, P], BF16)
    ones_in = inp("ones_in", [P, 1], F32R)
    onesr_in = inp("onesr_in", [1, P], F32R)
    onesrf_in = inp("onesrf_in", [1, P], F32)
    onesb_in = inp("onesb_in", [P, 1], BF16)
    onesrb_in = inp("onesrb_in", [1, P], BF16)
    qnw_in = inp("qnw_in", [P, HPC], F32)
    knw_in = inp("knw_in", [P, 1], F32)
    mask_in = inp("mask_in", [P, 4, 512], BF16)
    tri_in = inp("tri_in", [P, 2, TPC], F32R)        # tri[p,k,t] = (k*128+p) <= t
    iota_in = inp("iota_in", [P, CAP], F32)          # iota[p,c] = c
    rbbc_in = inp("rbbc_in", [P, E], F32)            # router bias, partition-replicated
    wg_p = inp("wg_p", [P, IMT, DKT, P], BF16)       # own expert only
    wu_p = inp("wu_p", [P, IMT, DKT, P], BF16)
    wd_p = inp("wd_p", [P, IKT, D], BF16)            # rhs layout [i-part, ki, d]

    out_sl = nc.dram_tensor("out_sl", [TPC, D], F32, kind="ExternalOutput")

    with tile.TileContext(nc) as tc, contextlib.ExitStack() as ctx:
        persist = ctx.enter_context(tc.tile_pool(name="persist", bufs=1))
        dram = ctx.enter_context(tc.tile_pool(name="dram", bufs=1, space="DRAM"))

        # ---------- persistent constants / long-lived small tiles ----------
        ones_sb = persist.tile([P, 1], F32R, tag="ones_sb")
        nc.sync.dma_start(ones_sb[:], ones_in.ap())
        ones_row = persist.tile([1, P], F32R, tag="ones_row")
        nc.sync.dma_start(ones_row[:], onesr_in.ap())
        ones_rowf = persist.tile([1, P], F32, tag="ones_rowf")
        nc.sync.dma_start(ones_rowf[:], onesrf_in.ap())
        idr_sb = persist.tile([P, P], F32R, tag="idr_sb")
        nc.sync.dma_start(idr_sb[:], id_r.ap())
        idf_sb = persist.tile([P, P], F32, tag="idf_sb")
        nc.sync.dma_start(idf_sb[:], id_f.ap())
        idb_sb = persist.tile([P, P], BF16, tag="idb_sb")
        nc.sync.dma_start(idb_sb[:], id_b.ap())
        ones_bf = persist.tile([P, 1], BF16, tag="ones_bf")
        nc.sync.dma_start(ones_bf[:], onesb_in.ap())
        ones_row_bf = persist.tile([1, P], BF16, tag="ones_row_bf")
        nc.sync.dma_start(ones_row_bf[:], onesrb_in.ap())
        qnw_sb = persist.tile([P, HPC], F32, tag="qnw_sb")
        nc.sync.dma_start(qnw_sb[:], qnw_in.ap())
        knw_sb = persist.tile([P, 1], F32, tag="knw_sb")
        nc.sync.dma_start(knw_sb[:], knw_in.ap())
        x1T = persist.tile([P, DMT, TPC], F32, tag="x1T")
        x1_tm = persist.tile([P, 2, D], F32, tag="x1_tm")
        h2tm_bf = persist.tile([P, 2, D], BF16, tag="h2tm_bf")
        W_all = persist.tile([P, NCORE, TPC], BF16, tag="W_all")

        with (
            tc.tile_pool(name="pXf", bufs=1) as pXf,     # xT_ownf: ph1-8
            tc.tile_pool(name="pAtt", bufs=1) as pAtt,   # qT/kT: ph3-6, v_tm: 5-6
        ):
            xT_ownf = pXf.tile([P, DKT, TPC], F32, tag="xT_ownf")
            mask_sb = pAtt.tile([P, 4, 512], BF16, tag="mask_sb")
            nc.sync.dma_start(mask_sb[:], mask_in.ap())

            # ---------- phase 1: transpose own x slice; ln1 rms row ----------
            with tc.tile_pool(name="p1", bufs=1) as p1:
                x_tm = p1.tile([P, 2, D], F32R, tag="x_tm")
                nc.sync.dma_start(x_tm[:], x_sl.ap().rearrange("(tb p) d -> p tb d", p=P))
                xT_own = p1.tile([P, DKT, TPC], BF16, tag="xT_own")
                with tc.tile_pool(name="tp_ps", bufs=2, space="PSUM") as tp_ps:
                    for kd in range(DKT):
                        for tb in range(2):
                            pt = tp_ps.tile([P, P], F32R, tag="tp")
                            nc.tensor.transpose(pt[:], x_tm[:, tb, kd * P:(kd + 1) * P],
                                                idr_sb[:])
                            nc.vector.tensor_copy(xT_own[:, kd, tb * P:(tb + 1) * P], pt[:])
                            nc.vector.tensor_copy(xT_ownf[:, kd, tb * P:(tb + 1) * P], pt[:])

                s_own = p1.tile([1, TPC], F32R, tag="s_own")
                with (
                    tc.tile_pool(name="sq_sb", bufs=3) as sq_pool,
                    tc.tile_pool(name="sq_ps", bufs=1, space="PSUM") as sq_ps,
                ):
                    acc = sq_ps.tile([1, TPC], F32, tag="sacc")
                    for kd in range(DKT):
                        sqf = sq_pool.tile([P, TPC], F32R, tag="sqf")
                        nc.vector.tensor_mul(sqf[:], xT_ownf[:, kd, :], xT_ownf[:, kd, :])
                        nc.tensor.matmul(acc[:], ones_sb[:], sqf[:],
                                         start=(kd == 0), stop=(kd == DKT - 1))
                    tmp = sq_pool.tile([1, TPC], F32, tag="stmp")
                    nc.vector.tensor_scalar(tmp[:], acc[:], 1.0 / D, EPS, OP.mult, OP.add)
                    nc.scalar.activation(tmp[:], tmp[:], AF.Sqrt)
                    nc.vector.reciprocal(tmp[:], tmp[:])
                    nc.vector.tensor_copy(s_own[:], tmp[:])

                # ---------- phase 2: local QKV over own tokens, all heads ----------
                # (weights streamed per 128-col tile; local full-dim qk sumsq)
                a2qkv_in = dram.tile([NCORE, 2 * P + 1, TPC], BF16, tag="a2qkv_in")
                a2qkv_out = dram.tile([NCORE, 2 * P + 1, TPC], BF16, tag="a2qkv_out")
                a2qq_in = dram.tile([NCORE, 2 * P + 1, TPC], BF16, tag="a2qq_in")
                a2qq_out = dram.tile([NCORE, 2 * P + 1, TPC], BF16, tag="a2qq_out")
                with (
                    tc.tile_pool(name="pQKV", bufs=1) as pQKV,
                    tc.tile_pool(name="qkv_w", bufs=3) as qkv_w,
                    tc.tile_pool(name="qkv_sq", bufs=3) as qsq_pool,
                    tc.tile_pool(name="qkv_ps", bufs=3, space="PSUM") as qkv_ps,
                    tc.tile_pool(name="qs_ps", bufs=1, space="PSUM") as qs_ps,
                ):
                    qkvT = pQKV.tile([P, 24, TPC], BF16, tag="qkvT")
                    qacc = qs_ps.tile([1, TPC], F32, tag="qacc")
                    kacc = qs_ps.tile([1, TPC], F32, tag="kacc")
                    cqo = pQKV.tile([1, TPC], BF16, tag="cqo")
                    cko = pQKV.tile([1, TPC], BF16, tag="cko")
                    crowp = pQKV
                    s2m = pQKV.tile([1, TPC], F32, tag="s2m")
                    nc.vector.tensor_mul(s2m[:], s_own[:], s_own[:])

                    def proj_tile(m):
                        wt = qkv_w.tile([P, DKT, P], BF16, tag="wt")
                        nc.sync.dma_start(wt[:], wqkv.ap()[:, m])
                        pt = qkv_ps.tile([P, TPC], F32, tag="qkvp")
                        for kd in range(DKT):
                            nc.tensor.matmul(pt[:], wt[:, kd, :], xT_own[:, kd, :],
                                             start=(kd == 0), stop=(kd == DKT - 1))
                        nc.vector.tensor_copy(qkvT[:, m, :], pt[:])
                        if m < 20:
                            sqf = qsq_pool.tile([P, TPC], F32R, tag="sqf")
                            nc.scalar.activation(sqf[:], pt[:], AF.Square)
                            dst = qacc if m < 16 else kacc
                            nc.tensor.matmul(dst[:], ones_sb[:], sqf[:],
                                             start=(m == 0 or m == 16),
                                             stop=(m == 15 or m == 19))

                    def rms_row(rowt, accp, mean_div, post):
                        t1 = crowp.tile([1, TPC], F32, tag="t1")
                        nc.vector.tensor_mul(t1[:], s2m[:], accp[:])
                        nc.vector.tensor_scalar(t1[:], t1[:], 1.0 / mean_div, EPS,
                                                OP.mult, OP.add)
                        nc.scalar.activation(t1[:], t1[:], AF.Sqrt)
                        nc.vector.reciprocal(t1[:], t1[:])
                        nc.vector.tensor_mul(t1[:], t1[:], s_own[:])
                        nc.vector.tensor_scalar_mul(rowt[:], t1[:], post)

                    # k/v first, shipped early so the a2a overlaps q projections
                    for m in range(16, 24):
                        proj_tile(m)
                    rms_row(cko, kacc, float(KVH * DH), 1.0)
                    for j in range(NCORE):
                        nc.sync.dma_start(a2qkv_in[j, 0:P, :], qkvT[:, 16 + j // 2, :])
                        nc.sync.dma_start(a2qkv_in[j, P:2 * P, :],
                                          qkvT[:, 20 + j // 2, :])
                        nc.sync.dma_start(a2qkv_in[j, 2 * P:2 * P + 1, :], cko[:])
                    nc.gpsimd.collective_compute("AllToAll", OP.bypass,
                                                 replica_groups=RG8,
                                                 ins=[a2qkv_in.opt()],
                                                 outs=[a2qkv_out.opt()])
                    for m in range(16):
                        proj_tile(m)
                    rms_row(cqo, qacc, float(H * DH), ISQ_DH)
                    for j in range(NCORE):
                        nc.sync.dma_start(
                            a2qq_in[j, 0:2 * P, :].rearrange("(m p) u -> p m u", p=P),
                            qkvT[:, 2 * j:2 * j + 2, :])
                        nc.sync.dma_start(a2qq_in[j, 2 * P:2 * P + 1, :], cqo[:])
                    nc.gpsimd.collective_compute("AllToAll", OP.bypass,
                                                 replica_groups=RG8,
                                                 ins=[a2qq_in.opt()],
                                                 outs=[a2qq_out.opt()])

            # ---------- phase 3b: assemble q/k/v + rms rows ----------
            qT = pAtt.tile([P, HPC, S], BF16, tag="qT")
            kT = pAtt.tile([P, S], BF16, tag="kT")
            with tc.tile_pool(name="pV", bufs=1) as pV:
                vT = pV.tile([P, S], BF16, tag="vT")
                with tc.tile_pool(name="p4", bufs=1) as p4:
                    cq = p4.tile([1, S], BF16, tag="cq")
                    ck = p4.tile([1, S], BF16, tag="ck")
                    for src in range(NCORE):
                        sl = slice(src * TPC, (src + 1) * TPC)
                        nc.sync.dma_start(kT[:, sl], a2qkv_out[src, 0:P, :])
                        nc.sync.dma_start(vT[:, sl], a2qkv_out[src, P:2 * P, :])
                        nc.sync.dma_start(ck[:, sl],
                                          a2qkv_out[src, 2 * P:2 * P + 1, :])
                    for src in range(NCORE):
                        sl = slice(src * TPC, (src + 1) * TPC)
                        nc.sync.dma_start(qT[:, 0, sl], a2qq_out[src, 0:P, :])
                        nc.sync.dma_start(qT[:, 1, sl], a2qq_out[src, P:2 * P, :])
                        nc.sync.dma_start(cq[:, sl],
                                          a2qq_out[src, 2 * P:2 * P + 1, :])

                    # ---------- phase 5: qk norms + rope; v to token-major ----------
                    with (
                        tc.tile_pool(name="bc", bufs=1) as bcp,
                        tc.tile_pool(name="bc_ps", bufs=2, space="PSUM") as bc_ps,
                        tc.tile_pool(name="rope", bufs=1) as rp2,
                    ):
                        rp = rp2
                        cos_sb = rp.tile([RD, S], BF16, tag="cos_sb")
                        nc.sync.dma_start(cos_sb[:], cos_in.ap())
                        sin_sb = rp.tile([RD, S], BF16, tag="sin_sb")
                        nc.sync.dma_start(sin_sb[:], sin_in.ap())
                        bq = bcp.tile([P, S], BF16, tag="bq")
                        bk = bcp.tile([P, S], BF16, tag="bk")
                        for (dst, row) in ((bq, cq), (bk, ck)):
                            for t4 in range(4):
                                bp = bc_ps.tile([P, 512], F32, tag="bp")
                                nc.tensor.matmul(bp[:], ones_row_bf[:],
                                                 row[:, t4 * 512:(t4 + 1) * 512],
                                                 start=True, stop=True)
                                nc.vector.tensor_copy(dst[:, t4 * 512:(t4 + 1) * 512],
                                                      bp[:])
                        for m in range(HPC):
                            nc.vector.tensor_mul(qT[:, m, :], qT[:, m, :], bq[:])
                            nc.vector.tensor_scalar_mul(qT[:, m, :], qT[:, m, :],
                                                        qnw_sb[:, m:m + 1])
                        nc.vector.tensor_mul(kT[:], kT[:], bk[:])
                        nc.vector.tensor_scalar_mul(kT[:], kT[:], knw_sb[:, 0:1])
                        HF = RD // 2
                        for ap_ in [qT[:, 0, :], qT[:, 1, :], kT[:]]:
                            # qsh = rotate_half layout: [q[HF:RD]; q[0:HF]]
                            qsh = rp.tile([RD, S], BF16, tag="qsh")
                            nc.sync.dma_start(qsh[0:HF, :], ap_[HF:RD, :])
                            nc.sync.dma_start(qsh[HF:RD, :], ap_[0:HF, :])
                            # sin table has rows 0:HF pre-negated on host
                            nc.vector.tensor_mul(qsh[0:RD, :], qsh[0:RD, :],
                                                 sin_sb[0:RD, :])
                            nc.vector.tensor_mul(ap_[0:RD, :], ap_[0:RD, :],
                                                 cos_sb[0:RD, :])
                            nc.vector.tensor_add(ap_[0:RD, :], ap_[0:RD, :],
                                                 qsh[0:RD, :])

                v_tm = pAtt.tile([P, DKT, DH], BF16, tag="v_tm")
                with tc.tile_pool(name="vt_ps", bufs=2, space="PSUM") as vt_ps:
                    for kt in range(DKT):
                        pt = vt_ps.tile([P, P], BF16, tag="vt")
                        nc.tensor.transpose(pt[:], vT[:, kt * P:(kt + 1) * P], idb_sb[:])
                        nc.vector.tensor_copy(v_tm[:, kt, :], pt[:])

            # ---------- phase 6: attention (exp over kt pairs) ----------
            with tc.tile_pool(name="pO", bufs=1) as pO:
                oT = pO.tile([P, HPC, S], BF16, tag="oT")
                with (
                    tc.tile_pool(name="sc_ps", bufs=2, space="PSUM") as sc_ps,
                    tc.tile_pool(name="o_ps", bufs=2, space="PSUM") as o_ps,
                    tc.tile_pool(name="sm_ps", bufs=1, space="PSUM") as sm_ps,
                    tc.tile_pool(name="eT", bufs=3) as e_pool,
                    tc.tile_pool(name="att_sb", bufs=2) as att_sb,
                ):
                    for m in range(HPC):
                        for qc in range(NCH):
                            nkt = 4 * qc + 4
                            qsl = slice(qc * 512, (qc + 1) * 512)
                            opsum = o_ps.tile([P, 512], F32, tag="o")
                            spsum = sm_ps.tile([1, 512], F32, tag="s")
                            for kt2 in range(nkt // 2):
                                scp = sc_ps.tile([P, 2, 512], F32, tag="sc")
                                for hh in range(2):
                                    kt = 2 * kt2 + hh
                                    nc.tensor.matmul(scp[:, hh, :],
                                                     kT[:, kt * P:(kt + 1) * P],
                                                     qT[:, m, qsl],
                                                     start=True, stop=True)
                                eT = e_pool.tile([P, 2, 512], BF16, tag="e")
                                nc.scalar.activation(eT[:], scp[:], AF.Exp)
                                for hh in range(2):
                                    kt = 2 * kt2 + hh
                                    if kt >= 4 * qc:
                                        nc.vector.tensor_mul(eT[:, hh, :], eT[:, hh, :],
                                                             mask_sb[:, kt - 4 * qc, :])
                                    nc.tensor.matmul(spsum[:], ones_bf[:], eT[:, hh, :],
                                                     start=(kt == 0),
                                                     stop=(kt == nkt - 1))
                                    nc.tensor.matmul(opsum[:], v_tm[:, kt, :],
                                                     eT[:, hh, :],
                                                     start=(kt == 0),
                                                     stop=(kt == nkt - 1))
                            rrow = att_sb.tile([1, 512], F32R, tag="rr")
                            with nc.allow_low_precision(
                                    reason="softmax denom reciprocal in f32r"):
                                nc.vector.reciprocal(rrow[:], spsum[:])
                            brp = sm_ps.tile([P, 512], F32, tag="brp")
                            nc.tensor.matmul(brp[:], ones_row[:], rrow[:],
                                             start=True, stop=True)
                            brb = att_sb.tile([P, 512], BF16, tag="brb")
                            nc.vector.tensor_copy(brb[:], brp[:])
                            nc.vector.tensor_copy(oT[:, m, qsl], opsum[:])
                            nc.vector.tensor_mul(oT[:, m, qsl], oT[:, m, qsl], brb[:])

                # ---------- phase 7: AllToAll o^T -> token-sharded ----------
                a2a_in = dram.tile([NCORE, HPC * P, TPC], BF16, tag="a2a_in")
                a2a_out = dram.tile([NCORE, HPC * P, TPC], BF16, tag="a2a_out")
                for j in range(NCORE):
                    nc.sync.dma_start(a2a_in[j].rearrange("(m p) u -> p m u", p=P),
                                      oT[:, :, j * TPC:(j + 1) * TPC])
                nc.gpsimd.collective_compute("AllToAll", OP.bypass, replica_groups=RG8,
                                             ins=[a2a_in.opt()], outs=[a2a_out.opt()])
            oTo = a2a_out.rearrange("r q u -> (r q) u")   # [H*DH, TPC] global odim rows

            # ---------- phase 8: o-proj + residual -> x1 (fp32) ----------
            with (
                tc.tile_pool(name="p8", bufs=1) as p8,
                tc.tile_pool(name="wo_str", bufs=3) as wo_str,
                tc.tile_pool(name="op_ps", bufs=2, space="PSUM") as op_ps,
            ):
                oTo_sb = p8.tile([P, DKT, TPC], BF16, tag="oTo_sb")
                nc.sync.dma_start(oTo_sb[:], oTo.rearrange("(ko p) u -> p ko u", p=P))
                for md in range(DMT):
                    wt = wo_str.tile([P, DKT, P], BF16, tag="wot")
                    nc.sync.dma_start(wt[:], wo_p.ap()[:, md])
                    pt = op_ps.tile([P, TPC], F32, tag="op")
                    for ko in range(DKT):
                        nc.tensor.matmul(pt[:], wt[:, ko, :], oTo_sb[:, ko, :],
                                         start=(ko == 0), stop=(ko == DKT - 1))
                    nc.vector.tensor_add(x1T[:, md, :], pt[:], xT_ownf[:, md, :])

        # ---------- MoE weight prefetch on the scalar HWDGE queue ----------
        moew = ctx.enter_context(tc.tile_pool(name="moew", bufs=1))
        wu_sb = moew.tile([P, IMT, DKT, P], BF16, tag="wu_sb")
        nc.scalar.dma_start(wu_sb[:], wu_p.ap())
        wd_sb = moew.tile([P, IKT, D], BF16, tag="wd_sb")
        nc.scalar.dma_start(wd_sb[:], wd_p.ap())

        # ---------- phase 9: ln2 rms, x1/h2 token-major, router, top-2 ----------
        with tc.tile_pool(name="pRoute", bufs=1) as pRoute:
            s2row = pRoute.tile([1, TPC], F32, tag="s2row")
            s2col = pRoute.tile([P, 2], F32, tag="s2col")
            posm1 = pRoute.tile([P, 2, E], F32, tag="posm1")
            indt = pRoute.tile([P, 2, E], F32, tag="indt")
            afft = pRoute.tile([P, 2, E], F32, tag="afft")
            with (
                tc.tile_pool(name="s2_sb", bufs=3) as s2_pool,
                tc.tile_pool(name="s2_ps", bufs=1, space="PSUM") as s2_ps,
                tc.tile_pool(name="tp9_ps", bufs=2, space="PSUM") as tp9_ps,
            ):
                acc2 = s2_ps.tile([1, TPC], F32, tag="acc2")
                for kd in range(DKT):
                    sqf = s2_pool.tile([P, TPC], F32R, tag="sqf")
                    nc.vector.tensor_mul(sqf[:], x1T[:, kd, :], x1T[:, kd, :])
                    nc.tensor.matmul(acc2[:], ones_sb[:], sqf[:],
                                     start=(kd == 0), stop=(kd == DKT - 1))
                nc.vector.tensor_scalar(s2row[:], acc2[:], 1.0 / D, EPS, OP.mult,
                                        OP.add)
                nc.scalar.activation(s2row[:], s2row[:], AF.Sqrt)
                nc.vector.reciprocal(s2row[:], s2row[:])

                # s2 as per-token columns via PE transpose; x1 token-major
                for tb in range(2):
                    sc_ps2 = tp9_ps.tile([P, 1], F32, tag="s2c")
                    nc.tensor.transpose(sc_ps2[:], s2row[0:1, tb * P:(tb + 1) * P],
                                        idf_sb[0:1, 0:1])
                    nc.vector.tensor_copy(s2col[:, tb:tb + 1], sc_ps2[:])
                for kd in range(DKT):
                    for tb in range(2):
                        pt = tp9_ps.tile([P, P], F32, tag="tp9")
                        nc.tensor.transpose(pt[:], x1T[:, kd, tb * P:(tb + 1) * P],
                                            idf_sb[:])
                        nc.scalar.activation(x1_tm[:, tb, kd * P:(kd + 1) * P],
                                             pt[:], AF.Copy)
                for tb in range(2):
                    h2f_tm = s2_pool.tile([P, D], F32, tag="h2f_tm")
                    nc.vector.tensor_scalar_mul(h2f_tm[:], x1_tm[:, tb, :],
                                                s2col[:, tb:tb + 1])
                    nc.scalar.activation(h2tm_bf[:, tb, :], h2f_tm[:], AF.Copy)

            # router logits (dim-major h2 in hi/lo fp32r for faithful top-2)
            with (
                tc.tile_pool(name="rt_sb", bufs=1) as rt_sb,
                tc.tile_pool(name="rt_sm", bufs=2) as rt_sm,
                tc.tile_pool(name="rt_ps", bufs=2, space="PSUM") as rt_ps,
                tc.tile_pool(name="rtw", bufs=1) as rtw,
                tc.tile_pool(name="rt_tmp", bufs=3) as rt_tmp,
            ):
                bs2p = rt_ps.tile([P, TPC], F32, tag="bs2p")
                nc.tensor.matmul(bs2p[:], ones_rowf[:], s2row[:], start=True, stop=True)
                bs2 = rt_sb.tile([P, TPC], F32, tag="bs2")
                nc.vector.tensor_copy(bs2[:], bs2p[:])
                h2h = rt_sb.tile([P, DKT, TPC], F32R, tag="h2h")
                h2l = rt_sb.tile([P, DKT, TPC], F32R, tag="h2l")
                for kd in range(DKT):
                    h2fk = rt_tmp.tile([P, TPC], F32, tag="h2fk")
                    nc.vector.tensor_mul(h2fk[:], x1T[:, kd, :], bs2[:])
                    nc.scalar.activation(h2h[:, kd, :], h2fk[:], AF.Copy)
                    nc.vector.tensor_sub(h2l[:, kd, :], h2fk[:], h2h[:, kd, :])
                rwh_sb = rtw.tile([P, DKT, E], F32R, tag="rwh_sb")
                nc.sync.dma_start(rwh_sb[:], rwh_in.ap())
                rwl_sb = rtw.tile([P, DKT, E], F32R, tag="rwl_sb")
                nc.sync.dma_start(rwl_sb[:], rwl_in.ap())
                lg = rt_ps.tile([E, TPC], F32, tag="lg")
                for kd in range(DKT):
                    nc.tensor.matmul(lg[:], rwh_sb[:, kd, :], h2h[:, kd, :],
                                     start=(kd == 0), stop=False)
                    nc.tensor.matmul(lg[:], rwh_sb[:, kd, :], h2l[:, kd, :],
                                     start=False, stop=False)
                    nc.tensor.matmul(lg[:], rwl_sb[:, kd, :], h2h[:, kd, :],
                                     start=False, stop=(kd == DKT - 1))
                lgs = rt_sb.tile([E, TPC], F32, tag="lgs")
                nc.vector.tensor_copy(lgs[:], lg[:])

                # top-2 select + affinity normalize, token-major
                rbbc_sb = rtw.tile([P, E], F32, tag="rbbc_sb")
                nc.sync.dma_start(rbbc_sb[:], rbbc_in.ap())
                tri_sb = rtw.tile([P, 2, TPC], F32R, tag="tri_sb")
                nc.sync.dma_start(tri_sb[:], tri_in.ap())
                indr = rt_sb.tile([P, 2, E], F32R, tag="indr")
                for tb in range(2):
                    lgT = rt_ps.tile([P, E], F32, tag="lgT")
                    nc.tensor.transpose(lgT[:], lgs[:, tb * P:(tb + 1) * P],
                                        idf_sb[0:E, 0:E])
                    sgt = rt_sm.tile([P, E], F32, tag="sgt")
                    nc.scalar.activation(sgt[:], lgT[:], AF.Sigmoid)
                    biased = rt_sm.tile([P, E], F32, tag="biased")
                    nc.vector.tensor_add(biased[:], sgt[:], rbbc_sb[:])
                    m1 = rt_sm.tile([P, 1], F32, tag="m1")
                    nc.vector.tensor_reduce(m1[:], biased[:], AX.X, OP.max)
                    eq = rt_sm.tile([P, E], F32, tag="eq")
                    nc.vector.tensor_scalar(eq[:], biased[:], m1[:, 0:1], -1e9,
                                            OP.is_equal, OP.mult)
                    t2 = rt_sm.tile([P, E], F32, tag="t2")
                    nc.vector.tensor_add(t2[:], biased[:], eq[:])
                    m2 = rt_sm.tile([P, 1], F32, tag="m2")
                    nc.vector.tensor_reduce(m2[:], t2[:], AX.X, OP.max)
                    nc.vector.tensor_scalar(indt[:, tb, :], biased[:], m2[:, 0:1],
                                            None, OP.is_ge)
                    aff0 = rt_sm.tile([P, E], F32, tag="aff0")
                    nc.vector.tensor_mul(aff0[:], sgt[:], indt[:, tb, :])
                    den = rt_sm.tile([P, 1], F32, tag="den")
                    nc.vector.tensor_reduce(den[:], aff0[:], AX.X, OP.add)
                    rden = rt_sm.tile([P, 1], F32, tag="rden")
                    nc.vector.reciprocal(rden[:], den[:])
                    nc.vector.tensor_scalar_mul(afft[:, tb, :], aff0[:], rden[:, 0:1])
                    nc.vector.tensor_copy(indr[:, tb, :], indt[:, tb, :])
                # slot position = cumulative count of routed tokens, minus 1
                for tb in range(2):
                    pos_ps = rt_ps.tile([P, E], F32, tag="pos_ps")
                    for kt in range(2):
                        nc.tensor.matmul(pos_ps[:], tri_sb[:, kt, tb * P:(tb + 1) * P],
                                         indr[:, kt, :], start=(kt == 0), stop=(kt == 1))
                    nc.vector.tensor_scalar_add(posm1[:, tb, :], pos_ps[:], -1.0)

            # ---------- phase 10a: gather + two half-capacity dispatch a2a ----------
            a2a1_in = [dram.tile([NCORE, D, HCAP], BF16, tag=f"a2a1_in{h}",
                                 name=f"a2a1_in{h}") for h in range(2)]
            a2a1_out = [dram.tile([NCORE, D, HCAP], BF16, tag=f"a2a1_out{h}",
                                  name=f"a2a1_out{h}") for h in range(2)]
            with (
                tc.tile_pool(name="g_one", bufs=1) as g_one,
                tc.tile_pool(name="g_sb", bufs=4) as g_sb,
                tc.tile_pool(name="g_ps", bufs=4, space="PSUM") as g_ps,
                tc.tile_pool(name="w_ps", bufs=2, space="PSUM") as w_ps,
                tc.tile_pool(name="disp", bufs=2) as disp_pool,
            ):
                iota_sb = g_one.tile([P, CAP], F32, tag="iota_sb")
                nc.sync.dma_start(iota_sb[:], iota_in.ap())
                # onehots for all (tb, halves, j)
                oneb_all = g_one.tile([P, 2, 2, NCORE, HCAP], BF16, tag="oneb_all")
                for tb in range(2):
                    for j in range(NCORE):
                        onef = g_sb.tile([P, CAP], F32, tag="onef")
                        nc.vector.tensor_scalar(onef[:], iota_sb[:],
                                                posm1[:, tb, j:j + 1], None,
                                                OP.is_equal)
                        nc.vector.tensor_scalar_mul(onef[:], onef[:],
                                                    indt[:, tb, j:j + 1])
                        for hs in range(2):
                            nc.vector.tensor_copy(oneb_all[:, tb, hs, j, :],
                                                  onef[:, hs * HCAP:(hs + 1) * HCAP])
                for hs in range(2):
                    dsb = disp_pool.tile([P, DKT, NCORE, HCAP], BF16, tag="dsb")
                    for kd in range(DKT):
                        for jg in range(2):
                            dp = g_ps.tile([P, 4 * HCAP], F32, tag="dp")
                            for tb in range(2):
                                rhs1 = oneb_all[:, tb, hs, 4 * jg:4 * jg + 4, :]
                                nc.tensor.matmul(dp[:],
                                                 h2tm_bf[:, tb, kd * P:(kd + 1) * P],
                                                 rhs1.rearrange("p j c -> p (j c)"),
                                                 start=(tb == 0), stop=(tb == 1))
                            dst = dsb[:, kd, 4 * jg:4 * jg + 4, :]
                            if kd % 2 == 0:
                                nc.vector.tensor_copy(
                                    dst.rearrange("p j c -> p (j c)"), dp[:])
                            else:
                                nc.scalar.activation(
                                    dst.rearrange("p j c -> p (j c)"), dp[:], AF.Copy)
                    for j in range(NCORE):
                        nc.sync.dma_start(
                            a2a1_in[hs][j].rearrange("(k p) c -> p k c", p=P),
                            dsb[:, :, j, :])
                    nc.gpsimd.collective_compute("AllToAll", OP.bypass,
                                                 replica_groups=RG8,
                                                 ins=[a2a1_in[hs].opt()],
                                                 outs=[a2a1_out[hs].opt()])
                # scatter weights (overlaps the dispatch a2a)
                for tb in range(2):
                    for j in range(NCORE):
                        onw = g_sb.tile([P, CAP], F32, tag="onw")
                        nc.vector.tensor_scalar(onw[:], iota_sb[:],
                                                posm1[:, tb, j:j + 1], None,
                                                OP.is_equal)
                        nc.vector.tensor_scalar_mul(onw[:], onw[:],
                                                    afft[:, tb, j:j + 1])
                        wp = w_ps.tile([P, P], F32, tag="wp")
                        nc.tensor.transpose(wp[:], onw[:], idf_sb[:])
                        nc.scalar.activation(W_all[:, j, tb * P:(tb + 1) * P],
                                             wp[:], AF.Copy)

        # ---------- phase 10b: own-expert GLU over received slots ----------
        a2a2_in = [dram.tile([NCORE, CAP, D // 2], BF16, tag=f"a2a2_in{h}",
                             name=f"a2a2_in{h}") for h in range(2)]
        a2a2_out = [dram.tile([NCORE, CAP, D // 2], BF16, tag=f"a2a2_out{h}",
                              name=f"a2a2_out{h}") for h in range(2)]
        with (
            tc.tile_pool(name="mlp_sb", bufs=1) as mlp_sb,
            tc.tile_pool(name="wmoe", bufs=3) as wmoe,
            tc.tile_pool(name="mlp_ps", bufs=2, space="PSUM") as mlp_ps,
            tc.tile_pool(name="mact", bufs=3) as mact,
        ):
            rhs_sb = mlp_sb.tile([P, DKT, SLOTS], BF16, tag="rhs_sb")
            for hs in range(2):
                for s in range(NCORE):
                    nc.sync.dma_start(
                        rhs_sb[:, :, hs * 512 + s * HCAP: hs * 512 + (s + 1) * HCAP],
                        a2a1_out[hs][s].rearrange("(k p) c -> p k c", p=P))
            act_sb = mlp_sb.tile([P, IMT, SLOTS], BF16, tag="act_sb")
            up_bf = mlp_sb.tile([P, SLOTS], BF16, tag="up_bf")
            for mi in range(IMT):
                wtg = wmoe.tile([P, DKT, P], BF16, tag="wmg")
                nc.sync.dma_start(wtg[:], wg_p.ap()[:, mi])
                for chs in range(2):
                    cs = slice(chs * 512, (chs + 1) * 512)
                    pt = mlp_ps.tile([P, 512], F32, tag="up")
                    for kd in range(DKT):
                        nc.tensor.matmul(pt[:], wu_sb[:, mi, kd, :], rhs_sb[:, kd, cs],
                                         start=(kd == 0), stop=(kd == DKT - 1))
                    nc.vector.tensor_copy(up_bf[:, cs], pt[:])
                for chs in range(2):
                    cs = slice(chs * 512, (chs + 1) * 512)
                    pt = mlp_ps.tile([P, 512], F32, tag="gate")
                    for kd in range(DKT):
                        nc.tensor.matmul(pt[:], wtg[:, kd, :], rhs_sb[:, kd, cs],
                                         start=(kd == 0), stop=(kd == DKT - 1))
                    gs = mact.tile([P, 512], BF16, tag="gs")
                    nc.scalar.activation(gs[:], pt[:], AF.Silu)
                    nc.vector.tensor_mul(gs[:], gs[:], up_bf[:, cs])
                    nc.vector.tensor_copy(act_sb[:, mi, cs], gs[:])
            for dh in range(2):
                for st in range(NCORE):     # slot tile: 128 slots = (hs, 2 srcs)
                    ret_bf = mact.tile([P, D // 2], BF16, tag="ret_bf")
                    for nch in range(2):
                        nsg = dh * 2 + nch
                        ns = slice(nsg * 512, (nsg + 1) * 512)
                        pt = mlp_ps.tile([P, 512], F32, tag="dn")
                        for ki in range(IKT):
                            nc.tensor.matmul(pt[:], act_sb[:, ki, st * P:(st + 1) * P],
                                             wd_sb[:, ki, ns],
                                             start=(ki == 0), stop=(ki == IKT - 1))
                        nc.vector.tensor_copy(ret_bf[:, nch * 512:(nch + 1) * 512],
                                              pt[:])
                    hs, sp = st // 4, (st % 4) * 2
                    nc.sync.dma_start(
                        a2a2_in[dh][sp, hs * HCAP:(hs + 1) * HCAP, :], ret_bf[0:64, :])
                    nc.sync.dma_start(
                        a2a2_in[dh][sp + 1, hs * HCAP:(hs + 1) * HCAP, :],
                        ret_bf[64:128, :])
                nc.gpsimd.collective_compute("AllToAll", OP.bypass, replica_groups=RG8,
                                             ins=[a2a2_in[dh].opt()],
                                             outs=[a2a2_out[dh].opt()])

        # ---------- phase 11: weighted scatter-add + residual; write output ----------
        with (
            tc.tile_pool(name="sc_sb", bufs=2) as scat_sb,
            tc.tile_pool(name="out_ps", bufs=1, space="PSUM") as out_psp,
            tc.tile_pool(name="out_sbp", bufs=1) as out_sbp,
        ):
            ops = [out_psp.tile([P, 512], F32, tag=f"ops{i}", name=f"ops{i}")
                   for i in range(8)]
            for dh in range(2):
                for j in range(NCORE):
                    rj = scat_sb.tile([P, D // 2], BF16, tag="rj")
                    nc.sync.dma_start(rj[:], a2a2_out[dh][j])
                    for tb in range(2):
                        for nch in range(2):
                            nsg = dh * 2 + nch
                            nc.tensor.matmul(ops[tb * 4 + nsg][:],
                                             W_all[:, j, tb * P:(tb + 1) * P],
                                             rj[:, nch * 512:(nch + 1) * 512],
                                             start=(j == 0), stop=(j == NCORE - 1))
            outt = out_sbp.tile([P, 2, D], F32, tag="outt")
            for tb in range(2):
                for nsg in range(4):
                    ns = slice(nsg * 512, (nsg + 1) * 512)
                    nc.vector.tensor_add(outt[:, tb, ns], ops[tb * 4 + nsg][:],
                                         x1_tm[:, tb, ns])
            nc.sync.dma_start(out_sl.ap().rearrange("(tb p) d -> p tb d", p=P),
                              outt[:])

    nc.compile()
    return nc


# ======================================================================
# host-side input preparation
# ======================================================================

def _trunc_hi(w, bits=12):
    """Zero all but the top `bits` mantissa bits (hi half survives fp32r rounding)."""
    u = np.ascontiguousarray(w, dtype=np.float32).view(np.uint32)
    mask = np.uint32(0xFFFFFFFF) << np.uint32(23 - bits)
    return (u & mask).view(np.float32)


def prep_in_maps(inputs):
    f32 = lambda a: np.ascontiguousarray(np.asarray(a), dtype=np.float32)
    x = f32(inputs["x"]).reshape(S, D)
    ln1 = f32(inputs["ln1_w"])
    ln2 = f32(inputs["ln2_w"])
    wq = f32(inputs["wq"]) * ln1[:, None]
    wk = f32(inputs["wk"]) * ln1[:, None]
    wv = f32(inputs["wv"]) * ln1[:, None]
    wo = f32(inputs["wo"])
    qnw = f32(inputs["qnorm_w"])
    knw = f32(inputs["knorm_w"])
    rw = f32(inputs["router_w"]) * ln2[:, None]
    rb = f32(inputs["router_bias"])
    wg = f32(inputs["wg"]) * ln2[None, :, None]
    wu = f32(inputs["wu"]) * ln2[None, :, None]
    wd = f32(inputs["wd"])

    pos = np.arange(S, dtype=np.float32)
    invf = (1.0 / (1e6 ** (np.arange(0, RD, 2, dtype=np.float32) / RD))).astype(np.float32)
    ang = pos[None, :] * invf[:, None]                      # [32, S]
    ang2 = np.concatenate([ang, ang], axis=0)               # [64, S]
    bf = ml_dtypes.bfloat16
    cos_t = np.cos(ang2).astype(bf)
    sin_t = np.sin(ang2).astype(np.float32)
    sin_t[:RD // 2] *= -1.0   # fold rotate_half sign into the table
    sin_t = sin_t.astype(bf)

    ident = np.eye(P, dtype=np.float32)
    ones_c = np.ones((P, 1), dtype=np.float32)
    p_i = np.arange(P)[:, None, None]
    off_i = np.arange(4)[None, :, None]
    q_i = np.arange(512)[None, None, :]
    mask = ((P * off_i + p_i) <= q_i).astype(bf)

    tri = (np.arange(TPC)[:, None] <= np.arange(TPC)[None, :]).astype(np.float32)
    tri_pk = np.ascontiguousarray(tri.reshape(2, P, TPC).transpose(1, 0, 2))
    iota_bc = np.broadcast_to(np.arange(CAP, dtype=np.float32), (P, CAP)).copy()
    rb_bc = np.broadcast_to(rb.reshape(1, E), (P, E)).copy()

    rwh = _trunc_hi(rw)
    rwl = (rw - rwh).astype(np.float32)
    pack_kd = lambda w: np.ascontiguousarray(
        w.reshape(DKT, P, w.shape[1]).transpose(1, 0, 2))   # [D, C] -> [P, DKT, C]

    wo_pk = np.ascontiguousarray(
        wo.reshape(DKT, P, DMT, P).transpose(1, 2, 0, 3).astype(bf))

    wfull = np.concatenate([wq, wk, wv], axis=1)        # [D, 3072]
    wqkv_pk = np.ascontiguousarray(
        wfull.reshape(DKT, P, 24, P).transpose(1, 2, 0, 3).astype(bf))

    in_maps = []
    for c in range(NCORE):
        qcols = slice(c * HPC * DH, (c + 1) * HPC * DH)
        kvcols = slice((c // 2) * DH, (c // 2 + 1) * DH)
        qnw_c = np.ascontiguousarray(qnw[qcols].reshape(HPC, P).T)
        knw_c = np.ascontiguousarray(knw[kvcols].reshape(1, P).T)
        wg_pk = np.ascontiguousarray(
            wg[c].reshape(DKT, P, IMT, P).transpose(1, 2, 0, 3).astype(bf))
        wu_pk = np.ascontiguousarray(
            wu[c].reshape(DKT, P, IMT, P).transpose(1, 2, 0, 3).astype(bf))
        wd_pk = np.ascontiguousarray(
            wd[c].reshape(IKT, P, D).transpose(1, 0, 2).astype(bf))
        in_maps.append({
            "x_sl": np.ascontiguousarray(x[c * TPC:(c + 1) * TPC]),
            "wqkv": wqkv_pk,
            "wo_p": wo_pk,
            "rwh_in": pack_kd(rwh),
            "rwl_in": pack_kd(rwl),
            "cos_in": cos_t,
            "sin_in": sin_t,
            "id_r": ident,
            "id_f": ident,
            "id_b": ident.astype(bf),
            "ones_in": ones_c,
            "onesr_in": np.ones((1, P), dtype=np.float32),
            "onesrf_in": np.ones((1, P), dtype=np.float32),
            "onesb_in": np.ones((P, 1), dtype=bf),
            "onesrb_in": np.ones((1, P), dtype=bf),
            "qnw_in": qnw_c,
            "knw_in": knw_c,
            "mask_in": mask,
            "tri_in": tri_pk,
            "iota_in": iota_bc,
            "rbbc_in": rb_bc,
            "wg_p": wg_pk,
            "wu_p": wu_pk,
            "wd_p": wd_pk,
        })
    return in_maps


_CACHE = {}


def get_module():
    if "nc" not in _CACHE:
        _CACHE["nc"] = build_module()
    return _CACHE["nc"]


def kernel(**inputs) -> np.ndarray:
    nc = get_module()
    in_maps = prep_in_maps(inputs)
    res = bass_utils.run_bass_kernel_spmd(nc, in_maps, core_ids=list(range(NCORE)))
    out = np.concatenate([res.results[c]["out_sl"] for c in range(NCORE)], axis=0)
    return out.reshape(1, S, D).astype(np.float32)


if __name__ == "__main__":
    build_module()
    print("module built ok")
